# revision 7
# baseline (speedup 1.0000x reference)
"""Trainium2 Bass kernel for nn_FMNet pixel-shuffle + sigmoid.

reference:  x = FV[:, 64:, :, :]                                 # [B, 64, 64, 64]
            out[b, 8i+r, 8j+c'] = sigmoid(x[b, 8r+c', i, j])     # [B, 1, 512, 512]

Per core (4 batches, pure data-parallel over batch):

Layout: partition p = (b:4, ch_q:4, i8:8); ch = 16*ch_q + 8*r2 + c',
i = 8*i8 + il.  tin free = (ch_lo:16, il:8, j:64); tout free =
(il:8, r2:2, j:64, c':8).  Output row = 64*i8 + 8*il + 2*ch_q + r2.

  - loads: 32 DMAs of 128 KiB per (b, r2, ch_q) with 2 KiB contiguous
    chunks (4x the baseline's 512 B -> ~24 vs ~19 GB/s per SDMA engine).
    Each DMA covers exactly one partition octet (SDMA engine 4*b + ch_q),
    so the 16 r2=0 loads span all 16 engines; issued r2=0-first across
    all three DGE streams (Sync, GpSimd, Scalar).
  - compute: 4 fused ScalarE ACTIVATE(Sigmoid) ops (r2 x il-half), each
    [128 x 2048], whose strided input AP performs the (c', j) ->
    (j*8 + c') interleave.  ACT(r2) needs only the r2 half of the
    channels, so ACT(0) overlaps the r2=1 load wave.  A dummy 1-element
    sigmoid pulls the 1.3 us ACT_TABLE_LOAD off the critical path.
  - stores: 32 DMAs of 128 KiB per (b, ch_q, il-half) with 4 KiB
    contiguous chunks (row pairs 2*ch_q, 2*ch_q+1), octet-balanced like
    the loads; A-half stores issue as soon as ACT(1,A) is done.

Total per-DMA-instruction descriptor-gen is ~0.65 us; 64 DMAs split
~25/25/14 across Sync/GpSimd/Scalar keeps every queue fed without any
stream exceeding the data phase.
"""

import os
import sys

if "/opt/trn_rl_repo" not in sys.path:
    sys.path.insert(0, "/opt/trn_rl_repo")

import numpy as np

import concourse.bass as bass
from concourse import mybir
from concourse.bass_utils import run_bass_kernel_spmd

N_CORES = 8
B = 32
B_LOC = B // N_CORES   # 4
H = W = 512
S = 64

LAST_EXEC_NS = None

_cached_nc = None


def _install_trace_hook():
    """Best-effort NTFF hook so BASS_TRACE=1 yields exec_time_ns."""
    try:
        import types

        import antenv

        try:
            from antenv.axon_hooks import get_axon_ntff_profile_hook  # noqa: F401

            return
        except ImportError:
            pass
        mod = types.ModuleType("antenv.axon_hooks")
        _state = {"hook": None}
        mod.set_axon_ntff_profile_hook = lambda h: _state.__setitem__("hook", h)
        mod.get_axon_ntff_profile_hook = lambda: _state["hook"]
        sys.modules["antenv.axon_hooks"] = mod
        antenv.axon_hooks = mod
        from trn_agent_boot.trn_boot import _ntff_profile_via_ctypes

        mod.set_axon_ntff_profile_hook(
            _ntff_profile_via_ctypes("/opt/axon/libaxon_pjrt.so")
        )
    except Exception:
        pass


def _build_nc():
    import contextlib

    F32 = mybir.dt.float32
    nc = bass.Bass("TRN2", num_devices=N_CORES)
    FV = nc.declare_dram_parameter("FV", [B_LOC, 128, S, S], F32, isOutput=False)
    OUT = nc.declare_dram_parameter("OUT", [B_LOC, W, H], F32, isOutput=True)

    tin = nc.alloc_sbuf_tensor("tin", [128, 8192], F32)
    tout = nc.alloc_sbuf_tensor("tout", [128, 8192], F32)
    scratch = nc.alloc_sbuf_tensor("scratch", [1, 8], F32)

    fv = FV[:]
    out = OUT[:]

    def load_aps(b, r2, ch_q):
        """128 KiB load of channels 16*ch_q + 8*r2 + [0,8) of batch b.

        src dims (i8:8, c':8, (il j):512); dst [8 p, c':8, 512]."""
        ch0 = 64 + 16 * ch_q + 8 * r2
        src = fv[b, ch0 : ch0 + 8].rearrange("c (i8 il) j -> i8 c (il j)", i8=8)
        pb = 32 * b + 8 * ch_q
        dst = tin.ap()[pb : pb + 8, 4096 * r2 : 4096 * (r2 + 1)].rearrange(
            "p (c v) -> p c v", c=8
        )
        return dst, src

    def act_aps(r2, h):
        """ACT slice (r2, il half h): [128, 4, 64, 8]."""
        in_v = (
            tin.ap()[:, 4096 * r2 : 4096 * (r2 + 1)]
            .rearrange("p (c il j) -> p il j c", c=8, il=8)[:, 4 * h : 4 * h + 4]
        )
        out_v = (
            tout.ap()
            .rearrange("p (il rr j c) -> p il rr j c", il=8, rr=2, j=64)[
                :, 4 * h : 4 * h + 4, r2
            ]
        )
        return out_v, in_v

    def store_aps(b, ch_q, h):
        """128 KiB store of rows 64*i8 + 8*il(+half h) + 2*ch_q + {0,1} of
        batch b; 4 KiB chunks (row pairs)."""
        dst = out[b].rearrange(
            "(i8 il cq rr) q -> i8 il cq (rr q)", i8=8, il=8, cq=4
        )[:, 4 * h : 4 * h + 4, ch_q, :]  # [8, 4, 1024]
        pb = 32 * b + 8 * ch_q
        src = tout.ap()[pb : pb + 8, :].rearrange("p (il v) -> p il v", il=8)[
            :, 4 * h : 4 * h + 4, :
        ]  # [8, 4, 1024]
        return dst, src

    # loads, r2=0 wave first; each (b, ch_q) pair is one partition octet
    sync_l0 = [(0, 0, q) for q in range(4)] + [(1, 0, 0), (1, 0, 1)]
    gp_l0 = [(2, 0, q) for q in range(4)] + [(3, 0, 0), (3, 0, 1)]
    sc_l0 = [(1, 0, 2), (1, 0, 3), (3, 0, 2), (3, 0, 3)]
    sync_l1 = [(0, 1, q) for q in range(4)] + [(1, 1, 0), (1, 1, 1)]
    gp_l1 = [(2, 1, q) for q in range(4)] + [(3, 1, 0), (3, 1, 1)]
    sc_l1 = [(1, 1, 2), (1, 1, 3), (3, 1, 2), (3, 1, 3)]

    # stores: (b, ch_q, half); A-half (h=0) as soon as ACT(1,A) done
    sync_sA = [(0, q, 0) for q in range(4)] + [(1, q, 0) for q in range(4)]
    gp_sA = [(2, q, 0) for q in range(4)] + [(3, q, 0) for q in range(4)]
    sync_sB = [(0, q, 1) for q in range(4)]
    gp_sB = [(2, q, 1) for q in range(4)]
    sc_sB = [(1, q, 1) for q in range(4)] + [(3, q, 1) for q in range(4)]

    with contextlib.ExitStack() as stack:
        block = stack.enter_context(nc.Block())
        # HWDGE (Sync/Scalar) and SWDGE (GpSimd) streams get separate
        # semaphores — a shared sem across the two DGE kinds is racy.
        sem_l0h = stack.enter_context(nc.semaphore("sem_l0h"))
        sem_l0s = stack.enter_context(nc.semaphore("sem_l0s"))
        sem_l1h = stack.enter_context(nc.semaphore("sem_l1h"))
        sem_l1s = stack.enter_context(nc.semaphore("sem_l1s"))
        sem_act = stack.enter_context(nc.semaphore("sem_act"))
        sem_oh = stack.enter_context(nc.semaphore("sem_oh"))
        sem_os = stack.enter_context(nc.semaphore("sem_os"))

        @block.sync
        def _(sync: bass.BassEngine):
            for b, r2, q in sync_l0:
                dst, src = load_aps(b, r2, q)
                sync.dma_start(out=dst, in_=src).then_inc(sem_l0h, 16)
            for b, r2, q in sync_l1:
                dst, src = load_aps(b, r2, q)
                sync.dma_start(out=dst, in_=src).then_inc(sem_l1h, 16)
            sync.wait_ge(sem_act, 3)  # ACT(0,A), ACT(0,B), ACT(1,A)
            for b, q, h in sync_sA:
                dst, src = store_aps(b, q, h)
                sync.dma_start(out=dst, in_=src).then_inc(sem_oh, 16)
            sync.wait_ge(sem_act, 4)  # + ACT(1,B)
            for b, q, h in sync_sB:
                dst, src = store_aps(b, q, h)
                sync.dma_start(out=dst, in_=src).then_inc(sem_oh, 16)
            sync.wait_ge(sem_oh, 20 * 16)
            sync.wait_ge(sem_os, 12 * 16)

        @block.gpsimd
        def _(g: bass.BassEngine):
            for b, r2, q in gp_l0:
                dst, src = load_aps(b, r2, q)
                g.dma_start(out=dst, in_=src).then_inc(sem_l0s, 16)
            for b, r2, q in gp_l1:
                dst, src = load_aps(b, r2, q)
                g.dma_start(out=dst, in_=src).then_inc(sem_l1s, 16)
            g.wait_ge(sem_act, 3)
            for b, q, h in gp_sA:
                dst, src = store_aps(b, q, h)
                g.dma_start(out=dst, in_=src).then_inc(sem_os, 16)
            g.wait_ge(sem_act, 4)
            for b, q, h in gp_sB:
                dst, src = store_aps(b, q, h)
                g.dma_start(out=dst, in_=src).then_inc(sem_os, 16)

        @block.scalar
        def _(scalar: bass.BassEngine):
            # dummy op to pull ACT_TABLE_LOAD (sigmoid) off the critical path
            scalar.activation(
                scratch.ap()[0:1, 0:1],
                nc.const_aps.tensor(0.0, (1, 1), mybir.dt.float32),
                mybir.ActivationFunctionType.Sigmoid,
            )
            for b, r2, q in sc_l0:
                dst, src = load_aps(b, r2, q)
                scalar.dma_start(out=dst, in_=src).then_inc(sem_l0h, 16)
            for b, r2, q in sc_l1:
                dst, src = load_aps(b, r2, q)
                scalar.dma_start(out=dst, in_=src).then_inc(sem_l1h, 16)
            scalar.wait_ge(sem_l0h, 10 * 16)
            scalar.wait_ge(sem_l0s, 6 * 16)
            for h in (0, 1):
                out_v, in_v = act_aps(0, h)
                scalar.activation(
                    out_v, in_v, mybir.ActivationFunctionType.Sigmoid
                ).then_inc(sem_act, 1)
            scalar.wait_ge(sem_l1h, 10 * 16)
            scalar.wait_ge(sem_l1s, 6 * 16)
            for h in (0, 1):
                out_v, in_v = act_aps(1, h)
                scalar.activation(
                    out_v, in_v, mybir.ActivationFunctionType.Sigmoid
                ).then_inc(sem_act, 1)
            scalar.wait_ge(sem_act, 4)
            for b, q, h in sc_sB:
                dst, src = store_aps(b, q, h)
                scalar.dma_start(out=dst, in_=src).then_inc(sem_oh, 16)

    return nc


def kernel(FV, batch_size=None, W=None, H=None, **_ignored):
    global _cached_nc, LAST_EXEC_NS
    FV = np.asarray(FV, dtype=np.float32)
    assert FV.shape == (B, 128, S, S), FV.shape

    trace = bool(os.environ.get("BASS_TRACE"))
    if trace:
        _install_trace_hook()

    if _cached_nc is None:
        _cached_nc = _build_nc()
    nc = _cached_nc

    in_maps = [{"FV": FV[k * B_LOC : (k + 1) * B_LOC]} for k in range(N_CORES)]
    res = None
    for attempt in range(3):
        try:
            res = run_bass_kernel_spmd(nc, in_maps, list(range(N_CORES)), trace=trace)
            break
        except Exception:
            # occasional transient NRT_EXEC_UNIT_UNRECOVERABLE on a cold
            # device; retry after a short pause
            if attempt == 2:
                raise
            import time

            time.sleep(2.0)
    if trace:
        LAST_EXEC_NS = res.exec_time_ns

    outs = [res.results[k]["OUT"] for k in range(N_CORES)]
    full = np.concatenate(outs, axis=0)  # [32, 512, 512]
    return full[:, None, :, :].astype(np.float32)


# revision 8
# speedup vs baseline: 1.9544x; 1.9544x over previous
"""Trainium2 Bass kernel for nn_FMNet pixel-shuffle + sigmoid.

reference:  x = FV[:, 64:, :, :]                                 # [B, 64, 64, 64]
            out[b, 8i+r, 8j+c'] = sigmoid(x[b, 8r+c', i, j])     # [B, 1, 512, 512]

Per core (4 batches, pure data-parallel over batch):

Layout: partition p = (b:4, i2:32), i = 2*i2 + ip.  tin free =
(c:64, ip:2, j:64); tout free = (ip:2, r:8, q:512), q = 8j + c'.
Output row = 16*i2 + 8*ip + 2*cq + rp where channel c = 8r + c',
r = 2*cq + rp (cq = channel quarter).

HWDGE SDMA-lane use scales with a DMA's partition span (8-partition
DMAs run on half the lanes at twice the per-lane load), so every DMA
here spans 32 partitions, like the proven baseline shape:

  - loads: 16 DMAs of 256 KiB per (b, cq) - channel quarter cq gives
    512 B chunks (i-row pairs).  Issued quarter-major on two DGE
    streams (Sync: b0/b1, GpSimd SWDGE: b2/b3), so quarter 0 is
    resident early and compute pipelines behind the load stream.
  - compute: 8 ScalarE ACTIVATE(Sigmoid) [128 x 1024] per (cq, ip),
    strided-read interleave (c', j) -> q = j*8+c' (innermost stride
    128 elems, the baseline-proven fast pattern).  ACT(cq,*) needs
    only quarter cq, so compute trails the per-quarter loads.
  - stores: 16 DMAs of 256 KiB per (b, cq) with 4 KiB chunks (row
    pairs 2cq, 2cq+1); issued as soon as ACT(cq, ip1) retires, so
    stores overlap the remaining load waves.

32 total DMA instructions (~0.65 us descriptor-gen each) split 16/16
across Sync/GpSimd keeps both queues fed; ScalarE only computes.
Separate semaphores per DGE kind (HWDGE vs SWDGE must not share).
"""

import os
import sys

if "/opt/trn_rl_repo" not in sys.path:
    sys.path.insert(0, "/opt/trn_rl_repo")

import numpy as np

import concourse.bass as bass
from concourse import mybir
from concourse.bass_utils import run_bass_kernel_spmd

N_CORES = 8
B = 32
B_LOC = B // N_CORES   # 4
H = W = 512
S = 64

LAST_EXEC_NS = None

_cached_nc = None


def _install_trace_hook():
    """Best-effort NTFF hook so BASS_TRACE=1 yields exec_time_ns."""
    try:
        import types

        import antenv

        try:
            from antenv.axon_hooks import get_axon_ntff_profile_hook  # noqa: F401

            return
        except ImportError:
            pass
        mod = types.ModuleType("antenv.axon_hooks")
        _state = {"hook": None}
        mod.set_axon_ntff_profile_hook = lambda h: _state.__setitem__("hook", h)
        mod.get_axon_ntff_profile_hook = lambda: _state["hook"]
        sys.modules["antenv.axon_hooks"] = mod
        antenv.axon_hooks = mod
        from trn_agent_boot.trn_boot import _ntff_profile_via_ctypes

        mod.set_axon_ntff_profile_hook(
            _ntff_profile_via_ctypes("/opt/axon/libaxon_pjrt.so")
        )
    except Exception:
        pass


def _build_nc():
    import contextlib

    F32 = mybir.dt.float32
    nc = bass.Bass("TRN2", num_devices=N_CORES)
    FV = nc.declare_dram_parameter("FV", [B_LOC, 128, S, S], F32, isOutput=False)
    OUT = nc.declare_dram_parameter("OUT", [B_LOC, W, H], F32, isOutput=True)

    tin = nc.alloc_sbuf_tensor("tin", [128, 8192], F32)
    tout = nc.alloc_sbuf_tensor("tout", [128, 8192], F32)
    scratch = nc.alloc_sbuf_tensor("scratch", [1, 8], F32)

    fv = FV[:]
    out = OUT[:]

    def load_aps(b, cq):
        """256 KiB load of channel quarter cq of batch b; [32 p] span."""
        ch0 = 64 + 16 * cq
        src = fv[b, ch0 : ch0 + 16].rearrange("c (i2 ip) j -> i2 c (ip j)", ip=2)
        dst = tin.ap()[32 * b : 32 * b + 32, 2048 * cq : 2048 * (cq + 1)].rearrange(
            "p (c v) -> p c v", c=16
        )
        return dst, src

    def act_aps(cq, ip):
        """ACT slice (channel quarter cq, row parity ip): [128, 2, 64, 8]."""
        in_v = tin.ap().rearrange(
            "p (rq rp cp ip j) -> p rq ip rp j cp", rq=4, rp=2, cp=8, ip=2
        )[:, cq, ip]
        out_v = tout.ap().rearrange(
            "p (ip rq rp j cp) -> p rq ip rp j cp", ip=2, rq=4, rp=2, j=64
        )[:, cq, ip]
        return out_v, in_v

    def store_aps(b, cq):
        """256 KiB store of rows 16*i2 + 8*ip + 2*cq + {0,1} of batch b."""
        dst = out[b].rearrange(
            "(i2 ip rq rp) q -> i2 ip rq (rp q)", i2=32, ip=2, rq=4
        )[:, :, cq, :]  # [32, 2, 1024]
        src = tout.ap()[32 * b : 32 * b + 32, :].rearrange(
            "p (ip rq v) -> p ip rq v", ip=2, rq=4
        )[:, :, cq, :]  # [32, 2, 1024]
        return dst, src

    with contextlib.ExitStack() as stack:
        block = stack.enter_context(nc.Block())
        # HWDGE (Sync) and SWDGE (GpSimd) streams must not share semaphores.
        sem_lh = [stack.enter_context(nc.semaphore(f"sem_lh{q}")) for q in range(4)]
        sem_ls = [stack.enter_context(nc.semaphore(f"sem_ls{q}")) for q in range(4)]
        sem_act = stack.enter_context(nc.semaphore("sem_act"))
        sem_oh = stack.enter_context(nc.semaphore("sem_oh"))
        sem_os = stack.enter_context(nc.semaphore("sem_os"))

        @block.sync
        def _(sync: bass.BassEngine):
            for cq in range(4):
                for b in (0, 1):
                    dst, src = load_aps(b, cq)
                    sync.dma_start(out=dst, in_=src).then_inc(sem_lh[cq], 16)
            for cq in range(4):
                sync.wait_ge(sem_act, 2 * (cq + 1))
                for b in (0, 1):
                    dst, src = store_aps(b, cq)
                    sync.dma_start(out=dst, in_=src).then_inc(sem_oh, 16)
            sync.wait_ge(sem_oh, 8 * 16)
            sync.wait_ge(sem_os, 8 * 16)

        @block.gpsimd
        def _(g: bass.BassEngine):
            for cq in range(4):
                for b in (2, 3):
                    dst, src = load_aps(b, cq)
                    g.dma_start(out=dst, in_=src).then_inc(sem_ls[cq], 16)
            for cq in range(4):
                g.wait_ge(sem_act, 2 * (cq + 1))
                for b in (2, 3):
                    dst, src = store_aps(b, cq)
                    g.dma_start(out=dst, in_=src).then_inc(sem_os, 16)

        @block.scalar
        def _(scalar: bass.BassEngine):
            # dummy op to pull ACT_TABLE_LOAD (sigmoid) off the critical path
            scalar.activation(
                scratch.ap()[0:1, 0:1],
                nc.const_aps.tensor(0.0, (1, 1), mybir.dt.float32),
                mybir.ActivationFunctionType.Sigmoid,
            )
            for cq in range(4):
                scalar.wait_ge(sem_lh[cq], 2 * 16)
                scalar.wait_ge(sem_ls[cq], 2 * 16)
                for ip in (0, 1):
                    out_v, in_v = act_aps(cq, ip)
                    scalar.activation(
                        out_v, in_v, mybir.ActivationFunctionType.Sigmoid
                    ).then_inc(sem_act, 1)

    return nc


def kernel(FV, batch_size=None, W=None, H=None, **_ignored):
    global _cached_nc, LAST_EXEC_NS
    FV = np.asarray(FV, dtype=np.float32)
    assert FV.shape == (B, 128, S, S), FV.shape

    trace = bool(os.environ.get("BASS_TRACE"))
    if trace:
        _install_trace_hook()

    if _cached_nc is None:
        _cached_nc = _build_nc()
    nc = _cached_nc

    in_maps = [{"FV": FV[k * B_LOC : (k + 1) * B_LOC]} for k in range(N_CORES)]
    res = None
    for attempt in range(3):
        try:
            res = run_bass_kernel_spmd(nc, in_maps, list(range(N_CORES)), trace=trace)
            break
        except Exception:
            # occasional transient NRT_EXEC_UNIT_UNRECOVERABLE on a cold
            # device; retry after a short pause
            if attempt == 2:
                raise
            import time

            time.sleep(2.0)
    if trace:
        LAST_EXEC_NS = res.exec_time_ns

    outs = [res.results[k]["OUT"] for k in range(N_CORES)]
    full = np.concatenate(outs, axis=0)  # [32, 512, 512]
    return full[:, None, :, :].astype(np.float32)
